# revision 31
# baseline (speedup 1.0000x reference)
"""Trainium2 Bass kernel for nn_CausalAggregator.

Computes, for target stocks y:
    out[y, :] = Beta[:, y] @ concat([X, adjacency[:, y, :]], 1) @ W + bias
              = (Beta.T @ X) @ Wf  +  (einsum('ny,nyc->yc', Beta, adj)) @ Wa + bias

Sharding: split Beta / adjacency along the target axis y across 8 cores;
replicate X, weight, bias. Each core computes 512 output rows; no
cross-device reduction.

Per-core algorithm (N=4096 source stocks, Y=512 targets, D=O=256, C=3).
The kernel is HBM-bound (DMA queues share one ~330 GB/s pool), so every
wire tensor is downcast host-side (free) to fp8-e4m3 and packed into ONE
stream: row n = [beta(512B) | x(256B) | adj channel-major(1536B)], grouped
GS=8 n-tiles per DMA with a partition-major permutation (contraction is
order-invariant in n).  ~9.5 MB/core vs 38 MB fp32.

All matmuls run in fp8 DoubleRow perf mode (K=256/pass, 0.5 cyc/row).
The einsum term runs on the PE as diagonal-block matmuls:
M[t,c] += Beta[:, yt]^T @ Adj_c[:, yt] accumulates [128,128] PSUM blocks
whose diagonals are the per-target sums; the epilogue extracts all
diagonals of channel c at once with a tiled-identity mask (DVE) and stacks
the three channel reduces into a [C, Y] psum tile via selector-matmuls.
PSUM start=True zeroes whole 2KB banks, so the interleaved per-slice M
accumulators are memset-initialized and accumulate with start=False.

Optional rx/rb flags add fp8 residual streams for X / Beta and one extra
DoubleRow cross-term pass  [X8|RX]^T @ [RB|B8]  per n-tile, cutting the
dominant quantization error ~6x for +3.1 MB of wire.

Constants and PSUM accumulators are hoisted out of the per-rep body so
back-to-back invocations pipeline: rep k+1's DMAs and matmuls overlap
rep k's epilogue, gated only by true data deps on the shared tiles.
"""

import numpy as np
import ml_dtypes

import concourse.mybir as mybir
import concourse.tile as tile
from concourse import bacc
from concourse.bass import ds, ts
from concourse.bass_utils import run_bass_kernel_spmd

P = 128
F32 = mybir.dt.float32
BF16 = mybir.dt.bfloat16
F8 = mybir.dt.float8e4
F16 = mybir.dt.float16
U8 = mybir.dt.uint8
DR = mybir.MatmulPerfMode.DoubleRow

# Full problem shapes (hardcoded; kernel.py must be self-contained).
N_FULL = 4096   # source stocks (contraction axis)
Y_TOTAL = 4096  # target stocks (sharded)
D_FULL = 256    # input features
O_FULL = 256    # output features
C_FULL = 3      # adjacency channels
N_CORES = 8
Y_FULL = Y_TOTAL // N_CORES  # per-core target slice
GS = 16         # n-tiles per DMA group
IO_BUFS = 3     # input stream double/triple buffering

OUT_F16 = False  # fp16 ExternalOutput crashes the exec unit; keep fp32
RX = False      # fp8 residual stream for X
RB = False      # fp8 residual stream for Beta


def _layout(Y, D, C, rx=RX, rb=RB):
    """Byte offsets of the packed subtile row [beta | x | adj | rx? | rb?]."""
    off, lay = 0, {}
    for name, sz, on in (("beta", Y, True), ("x", D, True), ("adj", C * Y, True),
                         ("rx", D, rx), ("rb", Y, rb)):
        if on:
            lay[name] = off
            off += sz
    return lay, off


def emit_body(tc, io, cst, N, Y, D, O, C, rx=RX, rb=RB):
    nc = tc.nc
    pkd, out = io["pkd"], io["out"]
    n_nt, n_yt, n_dt = N // P, Y // P, D // P
    lay, sub = _layout(Y, D, C, rx, rb)
    n_grp = n_nt // GS
    n_pairs_g = GS // 2

    iopool, fpool, opool = cst["iopool"], cst["fpool"], cst["opool"]
    gt_psum, m_psum, agg3 = cst["gt_psum"], cst["m_psum"], cst["agg3"]
    fp_pool = cst["fp_pool"]
    wf_t, wa_t, bias_bc, id_t, sel = (
        cst["wf_t"], cst["wa_t"], cst["bias_bc"], cst["id_t"], cst["sel"])

    gt_sb = [fpool.tile([P, Y], BF16, tag=f"gt{d_t}", name=f"gt{d_t}")
             for d_t in range(n_dt)]
    mask_sb = [fpool.tile([P, Y], BF16, tag=f"mask{c}", name=f"mask{c}")
               for c in range(C)]
    aggT = fpool.tile([C, Y], BF16, tag="aggT", name="aggT")

    for c in range(C):
        nc.vector.memset(m_psum[c], 0.0)

    queues = [nc.sync, nc.scalar]
    for g in range(n_grp):
        pk_t = iopool.tile([P, GS * sub], U8, tag="pk", name="pk")
        queues[g % 2].dma_start(out=pk_t, in_=pkd[ts(g, P), :])

        for jj in range(n_pairs_g):
            pi = g * n_pairs_g + jj
            first = pi == 0
            last = pi == n_grp * n_pairs_g - 1
            pair = pk_t[:, ds(jj * 2 * sub, 2 * sub)].rearrange(
                "p (i b) -> p i b", i=2)
            beta_p = pair[:, :, ds(lay["beta"], Y)].bitcast(F8)
            x_p = pair[:, :, ds(lay["x"], D)].bitcast(F8)
            adj_p = pair[:, :, ds(lay["adj"], C * Y)].bitcast(F8)

            for d_t in range(n_dt):
                nc.tensor.matmul(gt_psum[d_t], x_p[:, :, ts(d_t, P)],
                                 beta_p, start=first, stop=last and not (rx or rb),
                                 perf_mode=DR)
            for t in range(n_yt):
                for c in range(C):
                    nc.tensor.matmul(
                        m_psum[c][:, ts(t, P)],
                        beta_p[:, :, ts(t, P)],
                        adj_p[:, :, ds(c * Y + t * P, P)],
                        start=False, stop=last, perf_mode=DR,
                        skip_group_check=True)

            # residual cross-terms: (X8+RX)^T(B8+RB) ~= X8^T B8 + RX^T B8
            # + X8^T RB (RX^T RB dropped), each an extra DR pass over the
            # same subtile pairs
            extra = []
            if rx:
                rx_p = pair[:, :, ds(lay["rx"], D)].bitcast(F8)
                extra.append((rx_p, beta_p))
            if rb:
                rb_p = pair[:, :, ds(lay["rb"], Y)].bitcast(F8)
                extra.append((x_p, rb_p))
            for k, (lhs, rhs) in enumerate(extra):
                is_end = last and k == len(extra) - 1
                for d_t in range(n_dt):
                    nc.tensor.matmul(gt_psum[d_t], lhs[:, :, ts(d_t, P)],
                                     rhs, start=False, stop=is_end,
                                     perf_mode=DR, skip_group_check=True)

    # drain accumulators to SBUF
    for d_t in range(n_dt):
        nc.vector.tensor_copy(gt_sb[d_t], gt_psum[d_t])
    for c in range(C):
        nc.vector.tensor_mul(mask_sb[c], m_psum[c], id_t)

    # stack per-channel column-sums into agg3 [C, Y] via selector lhsT
    for c in range(C):
        nc.tensor.matmul(agg3, sel[:, ts(c, C)], mask_sb[c],
                         start=(c == 0), stop=(c == C - 1))
    nc.vector.tensor_copy(aggT, agg3)

    for y_t in range(n_yt):
        f_psum = fp_pool.tile([P, 2 * O], F32, tag="fpsum", name="fpsum")  # full bank
        fp = f_psum[:, 0:O]
        for d_t in range(n_dt):
            nc.tensor.matmul(fp, gt_sb[d_t][:, ts(y_t, P)],
                             wf_t[d_t], start=(d_t == 0), stop=False)
        nc.tensor.matmul(fp, aggT[:, ts(y_t, P)], wa_t,
                         start=False, stop=True)
        o_sb = opool.tile([P, O], F16 if OUT_F16 else F32, tag="osb", name="osb")
        nc.vector.tensor_add(o_sb, fp, bias_bc)
        (nc.scalar if y_t % 2 else nc.sync).dma_start(
            out=out[ts(y_t, P), :], in_=o_sb)


def emit_kernel(tc, io, N, Y, D, O, C, reps=1, rx=RX, rb=RB):
    nc = tc.nc
    n_dt = D // P
    with (
        tc.tile_pool(name="const", bufs=1) as cpool,
        tc.tile_pool(name="io", bufs=IO_BUFS) as iopool,
        tc.tile_pool(name="fin", bufs=2) as fpool,
        tc.tile_pool(name="osb", bufs=2) as opool,
        tc.tile_pool(name="acc", bufs=1, space="PSUM") as accpool,
        tc.tile_pool(name="fp", bufs=2, space="PSUM") as fp_pool,
    ):
        cst = {"iopool": iopool, "fpool": fpool, "opool": opool,
               "fp_pool": fp_pool}
        # sel[:, c*C+m] = (m==c): ones-reduce lhsT that stacks channel c's
        # column-sums into row c of a [C, Y] psum tile
        sel = cpool.tile([P, C * C], BF16, tag="sel", name="sel")
        nc.sync.dma_start(out=sel, in_=io["sel"])
        # [I I I I] tiled identity
        id_t = cpool.tile([P, Y], BF16, tag="ident", name="ident")
        nc.sync.dma_start(out=id_t, in_=io["ident"])
        wf_t = []
        for d_t in range(n_dt):
            t = cpool.tile([P, O], BF16, tag=f"wf{d_t}", name=f"wf{d_t}")
            nc.sync.dma_start(out=t, in_=io["wf"][ts(d_t, P), :])
            wf_t.append(t)
        wa_t = cpool.tile([C, O], BF16, tag="wa", name="wa")
        nc.sync.dma_start(out=wa_t, in_=io["wa"])
        bias_bc = cpool.tile([P, O], F32, tag="bias", name="bias")
        nc.sync.dma_start(out=bias_bc,
                          in_=io["bias"].unsqueeze(0).to_broadcast((P, O)))
        cst.update(wf_t=wf_t, wa_t=wa_t, bias_bc=bias_bc, id_t=id_t, sel=sel)

        # shared PSUM: 2 gt + 3 m + 1 agg3 + 2 f = 8 banks
        cst["gt_psum"] = [accpool.tile([P, Y], F32, tag=f"gtp{d}", name=f"gtp{d}")
                          for d in range(n_dt)]
        cst["m_psum"] = [accpool.tile([P, Y], F32, tag=f"mp{c}", name=f"mp{c}")
                         for c in range(C)]
        cst["agg3"] = accpool.tile([C, Y], F32, tag="agg3", name="agg3")

        for _ in range(reps):
            emit_body(tc, io, cst, N, Y, D, O, C, rx=rx, rb=rb)


def build_nc(N=N_FULL, Y=Y_FULL, D=D_FULL, O=O_FULL, C=C_FULL, reps=1,
             internal_inputs=False, rx=RX, rb=RB):
    nc = bacc.Bacc("TRN2", target_bir_lowering=False, debug=False)
    kind = "Internal" if internal_inputs else "ExternalInput"
    _, sub = _layout(Y, D, C, rx, rb)
    io = {
        "pkd": nc.dram_tensor("pkd", [N // GS, GS * sub], U8, kind=kind).ap(),
        "wf": nc.dram_tensor("wf", [D, O], BF16, kind=kind).ap(),
        "wa": nc.dram_tensor("wa", [C, O], BF16, kind=kind).ap(),
        "bias": nc.dram_tensor("bias", [O], F32, kind=kind).ap(),
        "ident": nc.dram_tensor("ident", [P, Y], BF16, kind=kind).ap(),
        "sel": nc.dram_tensor("sel", [P, C * C], BF16, kind=kind).ap(),
        "out": nc.dram_tensor("out", [Y, O], F16 if OUT_F16 else F32,
                              kind="ExternalOutput").ap(),
    }
    with tile.TileContext(nc) as tc:
        emit_kernel(tc, io, N, Y, D, O, C, reps=reps, rx=rx, rb=rb)
    nc.compile()
    return nc


_NC_CACHE = None


def _get_nc():
    global _NC_CACHE
    if _NC_CACHE is None:
        _NC_CACHE = build_nc()
    return _NC_CACHE


E4M3 = ml_dtypes.float8_e4m3


def _q8(a):
    return np.ascontiguousarray(a).astype(E4M3)


def run(adjacency, input_feature, Beta, weight, bias, trace=False):
    nc = _get_nc()
    adjacency = np.asarray(adjacency, dtype=np.float32)
    input_feature = np.asarray(input_feature, dtype=np.float32)
    Beta = np.asarray(Beta, dtype=np.float32)
    weight = np.ascontiguousarray(np.asarray(weight, dtype=np.float32))
    bias = np.ascontiguousarray(np.asarray(bias, dtype=np.float32))

    x8 = _q8(input_feature)
    wf = np.ascontiguousarray(weight[:D_FULL]).astype(ml_dtypes.bfloat16)
    wa = np.ascontiguousarray(weight[D_FULL:]).astype(ml_dtypes.bfloat16)
    ident = np.ascontiguousarray(
        np.tile(np.eye(P, dtype=ml_dtypes.bfloat16), (1, Y_FULL // P)))
    sel = np.zeros((P, C_FULL * C_FULL), dtype=ml_dtypes.bfloat16)
    for c in range(C_FULL):
        sel[:, c * C_FULL + c] = 1.0

    in_maps = []
    for i in range(N_CORES):
        ys = slice(i * Y_FULL, (i + 1) * Y_FULL)
        beta8 = _q8(Beta[:, ys])
        adj8 = _q8(adjacency[:, ys, :].transpose(0, 2, 1)).reshape(N_FULL, -1)
        parts = [beta8.view(np.uint8), x8.view(np.uint8), adj8.view(np.uint8)]
        if RX:
            parts.append(_q8(input_feature -
                             x8.astype(np.float32)).view(np.uint8))
        if RB:
            parts.append(_q8(Beta[:, ys] -
                             beta8.astype(np.float32)).view(np.uint8))
        row = np.concatenate(parts, axis=1)  # [N, sub]
        sub = row.shape[1]
        pkd = np.ascontiguousarray(
            row.reshape(N_FULL // (GS * P), GS, P, sub)
               .transpose(0, 2, 1, 3).reshape(N_FULL // GS, GS * sub))
        in_maps.append({
            "pkd": pkd,
            "wf": wf,
            "wa": wa,
            "bias": bias,
            "ident": ident,
            "sel": sel,
        })
    res = run_bass_kernel_spmd(nc, in_maps, core_ids=list(range(N_CORES)),
                               trace=trace)
    out = np.concatenate([res.results[i]["out"] for i in range(N_CORES)],
                         axis=0).astype(np.float32)
    return out, res


def kernel(adjacency, input_feature, Beta, weight, bias):
    out, _ = run(adjacency, input_feature, Beta, weight, bias, trace=False)
    return out
